# revision 22
# baseline (speedup 1.0000x reference)
"""MoE feed-forward (caption-conditioned top-2 routing) on 8 Trainium2 cores.

Strategy: data-parallel over batch (B=8 -> 1 sample/core). The router is a
~100 KFLOP computation (vs ~155 GFLOP for the expert FFNs), so it runs on the
host as part of input sharding: each core receives only its sample's two
selected experts' weights ("dispatch by topi" done at shard time). The
per-core device kernel is then a dense 2-expert FFN + weighted combine +
LayerNorm + residual:

  GEMM1  h.T[ff, s] = sum_d W1[d, ff] * x.T[d, s]   (PSUM fp32)
         g = gelu_tanh(h + b1)                      (fused bias+act, ACT)
  GEMM2  mixed[s, d] = sum_e sum_ff g_e.T[ff, s] * (topv_e * W2_e)[ff, d]
         (both experts accumulate into one PSUM tile; topv folded into W2)
  out    = x + LN(mixed + b2c) * gamma + beta       (bn_stats/bn_aggr + DVE)

Matmuls run as float32r (fp32 data, FP22 multiply): full bf16-rate on the PE
array with ~13-bit mantissa precision. S is processed in halves of 512 so the
8 PSUM banks cover GEMM1 accumulation (round-robin) and 8 live mixed tiles.
"""

import sys

if "/opt/trn_rl_repo" not in sys.path:
    sys.path.insert(0, "/opt/trn_rl_repo")

import numpy as np

import concourse.bass as bass
import concourse.mybir as mybir
import concourse.tile as tile
from concourse.bass_utils import run_bass_kernel_spmd

# ---- problem constants (hardcoded per spec) ----
B, S, D = 8, 1024, 768
FF = 4 * D  # 3072
E = 8
TOPK = 2
LN_EPS = 1e-5
N_CORES = 8

KD = D // 128   # 6  k-tiles over D
KF = FF // 128  # 24 k-tiles over FF
SH = S // 2     # 512 s-half
SC = SH // 128  # 4  s-chunks per half

F32 = mybir.dt.float32
F16 = mybir.dt.float16

# ----------------------------------------------------------------------------
# This container's walrus build only accepts ONE sync wait per instruction
# (setupSyncWait "Too many sync wait commands"), while Tile routinely attaches
# several (e.g. a matmul waiting on two DMA queues, the tail drain waiting on
# every live semaphore). Post-pass: hoist surplus waits onto same-engine nops
# inserted immediately before the instruction — the engine executes its queue
# in order, so blocking on the nops first is semantically identical.
_MAX_INST_WAITS = 1


def _split_multi_waits(nc):
    for fn in nc.m.functions:
        for bb in fn.blocks:
            out = []
            for inst in bb.instructions:
                si = inst.sync_info
                if si is not None and len(si.on_wait) > _MAX_INST_WAITS:
                    waits = list(si.on_wait)
                    surplus, keep = waits[:-_MAX_INST_WAITS], waits[-_MAX_INST_WAITS:]
                    for w in surplus:
                        nop = mybir.InstNoOp(
                            name=nc.get_next_instruction_name(),
                            engine=inst.engine,
                            ins=[],
                            outs=[],
                            sync_info=mybir.SyncInfo(on_wait=[w], on_update=[]),
                        )
                        out.append(nop)
                    inst.sync_info = mybir.SyncInfo(
                        on_wait=keep, on_update=list(si.on_update)
                    )
                out.append(inst)
            bb.instructions[:] = out
# ----------------------------------------------------------------------------


def _r(ap):
    """Reinterpret an fp32 AP as float32r for the PE (FP22 multiply path)."""
    return ap.bitcast(F32R)


def _build_kernel():
    nc = bass.Bass("TRN2", target_bir_lowering=False, debug=False,
                   num_devices=N_CORES)

    xt = nc.dram_tensor("xt", [128, KD, S], F16, kind="ExternalInput")
    xr = nc.dram_tensor("xr", [S, D], F32, kind="ExternalInput")
    w1t = nc.dram_tensor("w1t", [TOPK, KF, 128, KD, 128], F16,
                         kind="ExternalInput")
    w2t = nc.dram_tensor("w2t", [TOPK, 128, KF, D], F16, kind="ExternalInput")
    b1t = nc.dram_tensor("b1t", [128, TOPK, KF], F32, kind="ExternalInput")
    b2c = nc.dram_tensor("b2c", [128, D], F32, kind="ExternalInput")
    gam = nc.dram_tensor("gam", [128, D], F32, kind="ExternalInput")
    out = nc.dram_tensor("out", [S, D], F32, kind="ExternalOutput")

    gelu = mybir.ActivationFunctionType.Gelu_apprx_tanh

    with tile.TileContext(nc) as tc:
        with (
            tc.tile_pool(name="consts", bufs=1) as consts,
            tc.tile_pool(name="w1p", bufs=12) as w1p,
            tc.tile_pool(name="w2p", bufs=8) as w2p,
            tc.tile_pool(name="ps", bufs=4, space="PSUM") as psp,
            tc.tile_pool(name="xrp", bufs=4) as xrp,
            tc.tile_pool(name="mixp", bufs=2) as mixp,
            tc.tile_pool(name="outp", bufs=3) as outp,
            tc.tile_pool(name="stat", bufs=4) as stat,
        ):
            # ---- constants / persistent tiles ----
            xt_sb = consts.tile([128, KD, S], F16)
            b1_sb = consts.tile([128, TOPK, KF], F32)
            # b2c/gam are only needed by the LayerNorm stage — allocated here,
            # but their DMAs are issued mid-GEMM1 (below) so they don't delay
            # the startup-critical w1 stream on the gpsimd queue.
            b2c_sb = consts.tile([128, D], F32)
            gam_sb = consts.tile([128, D], F32)
            eps_sb = consts.tile([128, 1], F32)
            nc.vector.memset(eps_sb[:], LN_EPS)
            # g = gelu(x @ W1 + b1), transposed layout [ff, s-half]
            gt = consts.tile([128, TOPK, KF, SH], F16)

            for half in range(2):
                s0 = half * SH

                # ---- GEMM1: h.T = W1.T-tiles @ x.T, fused bias+gelu ----
                # m-tiles processed in pairs with MMs interleaved between two
                # PSUM tiles: consecutive matmuls then hit different banks,
                # which lets fill/drain overlap (same-bank back-to-back
                # accumulation serializes on the bank write port).
                for e in range(TOPK):
                    for m in range(0, KF, 2):
                        # the first few tiles of the kernel ride the (idle,
                        # low-latency) HWDGE queue so GEMM1 starts promptly;
                        # the steady stream stays on SWDGE
                        weng = (nc.sync if half == 0 and e == 0 and m < 8
                                else nc.gpsimd)
                        w1a = w1p.tile([128, KD, 128], F16, tag="w1")
                        weng.dma_start(w1a[:], w1t[e, m])
                        w1b = w1p.tile([128, KD, 128], F16, tag="w1")
                        weng.dma_start(w1b[:], w1t[e, m + 1])
                        if half == 0 and e == 0 and m == 0:
                            for kd in range(KD):
                                nc.sync.dma_start(
                                    xt_sb[:, kd, 0:SH], xt[:, kd, 0:SH])
                            nc.sync.dma_start(b1_sb[:], b1t[:])
                        if half == 0 and e == 0 and m == 2:
                            for kd in range(KD):
                                nc.sync.dma_start(
                                    xt_sb[:, kd, SH:S], xt[:, kd, SH:S])
                        pha_t = psp.tile([128, 2 * SH], F32, tag="ps")
                        phb_t = psp.tile([128, 2 * SH], F32, tag="ps")
                        pha = pha_t[:, :SH]
                        phb = phb_t[:, :SH]
                        for kd in range(KD):
                            nc.tensor.matmul(
                                pha,
                                w1a[:, kd, :],
                                xt_sb[:, kd, s0 : s0 + SH],
                                start=(kd == 0),
                                stop=(kd == KD - 1),
                            )
                            nc.tensor.matmul(
                                phb,
                                w1b[:, kd, :],
                                xt_sb[:, kd, s0 : s0 + SH],
                                start=(kd == 0),
                                stop=(kd == KD - 1),
                            )
                        nc.scalar.activation(
                            out=gt[:, e, m, :],
                            in_=pha,
                            func=gelu,
                            bias=b1_sb[:, e, m : m + 1],
                            scale=1.0,
                        )
                        nc.scalar.activation(
                            out=gt[:, e, m + 1, :],
                            in_=phb,
                            func=gelu,
                            bias=b1_sb[:, e, m + 1 : m + 2],
                            scale=1.0,
                        )
                        if half == 0 and e == 0 and m == 8:
                            # LN constants: sync queue is idle mid-GEMM1
                            nc.sync.dma_start(b2c_sb[:], b2c[:])
                            nc.sync.dma_start(gam_sb[:], gam[:])

                # ---- GEMM2: mixed[s,d] accumulates both experts ----
                # One [128, 1024] PSUM tile (2 banks) per s-chunk: cols 0:512
                # (bank A) and 512:768 (bank B) are separate accumulation
                # groups; LN later reads 0:768 as one contiguous slab.
                pm = [psp.tile([128, 2 * SH], F32, tag="ps", name=f"pm_{half}_{i}")
                      for i in range(SC)]
                xr_sbs = []
                for sc in range(SC):
                    srow = s0 + sc * 128
                    xr_sb = xrp.tile([128, D], F32, tag="xr")
                    nc.gpsimd.dma_start(xr_sb[:], xr[srow : srow + 128, :])
                    xr_sbs.append(xr_sb)
                for e in range(TOPK):
                    for kf in range(KF):
                        w2_sb = w2p.tile([128, D], F16, tag="w2")
                        nc.sync.dma_start(w2_sb[:], w2t[e, :, kf, :])
                        first = e == 0 and kf == 0
                        last = e == TOPK - 1 and kf == KF - 1
                        for sc in range(SC):
                            lhsT = gt[:, e, kf, sc * 128 : (sc + 1) * 128]
                            nc.tensor.matmul(
                                pm[sc][:, 0:512],
                                lhsT,
                                w2_sb[:, 0:512],
                                start=first,
                                stop=last,
                            )
                            nc.tensor.matmul(
                                pm[sc][:, 512:768],
                                lhsT,
                                w2_sb[:, 512:768],
                                start=first,
                                stop=last,
                            )

                # ---- combine + LayerNorm + residual, per s-chunk ----
                for sc in range(SC):
                    srow = s0 + sc * 128
                    xr_sb = xr_sbs[sc]
                    mixed = mixp.tile([128, D], F32, tag="mx")
                    nc.vector.tensor_add(mixed[:], pm[sc][:, 0:768],
                                         b2c_sb[:])

                    stats = stat.tile([128, 3, 6], F32, tag="st")
                    for i in range(3):
                        nc.vector.bn_stats(
                            out=stats[:, i, :],
                            in_=mixed[:, i * 256 : (i + 1) * 256],
                        )
                    mv = stat.tile([128, 2], F32, tag="mv")
                    nc.vector.bn_aggr(out=mv[:], in_=stats[:])

                    rs = stat.tile([128, 1], F32, tag="rs")
                    nc.scalar.activation(
                        out=rs[:], in_=mv[:, 1:2],
                        func=mybir.ActivationFunctionType.Sqrt,
                        bias=eps_sb[:], scale=1.0,
                    )
                    nc.vector.reciprocal(out=rs[:], in_=rs[:])

                    # (mixed - mu) * rs, then * gamma, then + (x + beta)
                    nc.vector.tensor_scalar(
                        out=mixed[:],
                        in0=mixed[:],
                        scalar1=mv[:, 0:1],
                        scalar2=rs[:],
                        op0=mybir.AluOpType.subtract,
                        op1=mybir.AluOpType.mult,
                    )
                    o_sb = outp.tile([128, D], F32, tag="o")
                    nc.vector.tensor_mul(o_sb[:], mixed[:], gam_sb[:])
                    nc.gpsimd.tensor_add(o_sb[:], o_sb[:], xr_sb[:])
                    nc.sync.dma_start(out[srow : srow + 128, :], o_sb[:])

    _split_multi_waits(nc)
    return nc


_NC_CACHE = None


def _get_nc():
    global _NC_CACHE
    if _NC_CACHE is None:
        _NC_CACHE = _build_kernel()
    return _NC_CACHE


def _route(x, text_state, router_w, router_b):
    """Host router replicating the jax fp32 ops."""
    x = x.astype(np.float32, copy=False)
    pooled = x.mean(axis=1, dtype=np.float32)                 # [B, D]
    feat = np.concatenate([pooled, text_state.astype(np.float32)], axis=-1)
    logits = feat @ router_w.astype(np.float32) + router_b.astype(np.float32)
    m = logits.max(axis=-1, keepdims=True)
    p = np.exp(logits - m)
    probs = (p / p.sum(axis=-1, keepdims=True)).astype(np.float32)
    # jax.lax.top_k: descending, ties -> lower index first
    topi = np.argsort(-probs, axis=-1, kind="stable")[:, :TOPK]
    topv = np.take_along_axis(probs, topi, axis=-1)
    return probs, topi, topv


def kernel(x, text_state, W1, b1, W2, b2, router_w, router_b, ln_gamma,
           ln_beta):
    x = np.asarray(x, dtype=np.float32)
    text_state = np.asarray(text_state, dtype=np.float32)
    W1 = np.asarray(W1, dtype=np.float32)
    b1 = np.asarray(b1, dtype=np.float32)
    W2 = np.asarray(W2, dtype=np.float32)
    b2 = np.asarray(b2, dtype=np.float32)
    router_w = np.asarray(router_w, dtype=np.float32)
    router_b = np.asarray(router_b, dtype=np.float32)
    ln_gamma = np.asarray(ln_gamma, dtype=np.float32)
    ln_beta = np.asarray(ln_beta, dtype=np.float32)

    probs, topi, topv = _route(x, text_state, router_w, router_b)

    gam_b = np.ascontiguousarray(np.broadcast_to(ln_gamma, (128, D)))

    in_maps = []
    for bidx in range(B):
        xb = x[bidx]                                          # [S, D]
        # x.T tiled: xt[p, kd, s] = x[s, kd*128+p]
        xt = np.ascontiguousarray(
            xb.T.reshape(KD, 128, S).transpose(1, 0, 2)).astype(np.float16)
        xr = np.ascontiguousarray(xb + ln_beta)               # residual + beta
        eids = topi[bidx]
        # w1t[e, m, p, kd, j] = W1[eid, kd*128+p, m*128+j]
        w1g = W1[eids]                                        # [2, D, FF]
        w1t = np.ascontiguousarray(
            w1g.reshape(TOPK, KD, 128, KF, 128).transpose(0, 3, 2, 1, 4)
        ).astype(np.float16)
        # w2t[e, p, kf, d] = topv_e * W2[eid, kf*128+p, d]
        w2g = W2[eids] * topv[bidx][:, None, None]            # [2, FF, D]
        w2t = np.ascontiguousarray(
            w2g.reshape(TOPK, KF, 128, D).transpose(0, 2, 1, 3)
        ).astype(np.float16)
        # b1t[p, e, m] = b1[eid, m*128+p]
        b1g = b1[eids].reshape(TOPK, KF, 128)                 # [2, KF, 128]
        b1t = np.ascontiguousarray(b1g.transpose(2, 0, 1))
        # combined, topv-weighted b2, broadcast over partitions
        b2cv = (topv[bidx][:, None] * b2[eids]).sum(axis=0)   # [D]
        b2cb = np.ascontiguousarray(np.broadcast_to(b2cv, (128, D)))

        in_maps.append({
            "xt": xt, "xr": xr, "w1t": w1t, "w2t": w2t,
            "b1t": b1t, "b2c": b2cb, "gam": gam_b,
        })

    global _last_in_maps
    _last_in_maps = in_maps

    nc = _get_nc()
    res = run_bass_kernel_spmd(nc, in_maps, core_ids=list(range(N_CORES)))
    out = np.stack([res.results[bidx]["out"] for bidx in range(B)], axis=0)
    return out, probs


if __name__ == "__main__":
    rng = np.random.default_rng(0)
    inputs = {
        "x": rng.standard_normal((B, S, D), dtype=np.float32),
        "text_state": rng.standard_normal((B, D), dtype=np.float32),
        "W1": rng.standard_normal((E, D, FF), dtype=np.float32) * 0.02,
        "b1": np.zeros((E, FF), np.float32),
        "W2": rng.standard_normal((E, FF, D), dtype=np.float32) * 0.02,
        "b2": np.zeros((E, D), np.float32),
        "router_w": rng.standard_normal((2 * D, E), dtype=np.float32) * 0.02,
        "router_b": np.zeros((E,), np.float32),
        "ln_gamma": np.ones((D,), np.float32),
        "ln_beta": np.zeros((D,), np.float32),
    }
    o, pr = kernel(**inputs)
    print(o.shape, pr.shape, o.dtype, pr.dtype)


# revision 23
# speedup vs baseline: 1.0363x; 1.0363x over previous
"""MoE feed-forward (caption-conditioned top-2 routing) on 8 Trainium2 cores.

Strategy: data-parallel over batch (B=8 -> 1 sample/core). The router is a
~100 KFLOP computation (vs ~155 GFLOP for the expert FFNs), so it runs on the
host as part of input sharding: each core receives only its sample's two
selected experts' weights ("dispatch by topi" done at shard time). The
per-core device kernel is then a dense 2-expert FFN + weighted combine +
LayerNorm + residual:

  GEMM1  h.T[ff, s] = sum_d W1[d, ff] * x.T[d, s]   (PSUM fp32)
         g = gelu_tanh(h + b1)                      (fused bias+act, ACT)
  GEMM2  mixed[s, d] = sum_e sum_ff g_e.T[ff, s] * (topv_e * W2_e)[ff, d]
         (both experts accumulate into one PSUM tile; topv folded into W2)
  out    = x + LN(mixed + b2c) * gamma + beta       (bn_stats/bn_aggr + DVE)

Matmuls run as float32r (fp32 data, FP22 multiply): full bf16-rate on the PE
array with ~13-bit mantissa precision. S is processed in halves of 512 so the
8 PSUM banks cover GEMM1 accumulation (round-robin) and 8 live mixed tiles.
"""

import sys

if "/opt/trn_rl_repo" not in sys.path:
    sys.path.insert(0, "/opt/trn_rl_repo")

import numpy as np

import concourse.bass as bass
import concourse.mybir as mybir
import concourse.tile as tile
from concourse.bass_utils import run_bass_kernel_spmd

# ---- problem constants (hardcoded per spec) ----
B, S, D = 8, 1024, 768
FF = 4 * D  # 3072
E = 8
TOPK = 2
LN_EPS = 1e-5
N_CORES = 8

KD = D // 128   # 6  k-tiles over D
KF = FF // 128  # 24 k-tiles over FF
SH = S // 2     # 512 s-half
SC = SH // 128  # 4  s-chunks per half

F32 = mybir.dt.float32
F16 = mybir.dt.float16

# ----------------------------------------------------------------------------
# This container's walrus build only accepts ONE sync wait per instruction
# (setupSyncWait "Too many sync wait commands"), while Tile routinely attaches
# several (e.g. a matmul waiting on two DMA queues, the tail drain waiting on
# every live semaphore). Post-pass: hoist surplus waits onto same-engine nops
# inserted immediately before the instruction — the engine executes its queue
# in order, so blocking on the nops first is semantically identical.
_MAX_INST_WAITS = 1


def _split_multi_waits(nc):
    for fn in nc.m.functions:
        for bb in fn.blocks:
            out = []
            for inst in bb.instructions:
                si = inst.sync_info
                if si is not None and len(si.on_wait) > _MAX_INST_WAITS:
                    waits = list(si.on_wait)
                    surplus, keep = waits[:-_MAX_INST_WAITS], waits[-_MAX_INST_WAITS:]
                    for w in surplus:
                        nop = mybir.InstNoOp(
                            name=nc.get_next_instruction_name(),
                            engine=inst.engine,
                            ins=[],
                            outs=[],
                            sync_info=mybir.SyncInfo(on_wait=[w], on_update=[]),
                        )
                        out.append(nop)
                    inst.sync_info = mybir.SyncInfo(
                        on_wait=keep, on_update=list(si.on_update)
                    )
                out.append(inst)
            bb.instructions[:] = out
# ----------------------------------------------------------------------------


def _r(ap):
    """Reinterpret an fp32 AP as float32r for the PE (FP22 multiply path)."""
    return ap.bitcast(F32R)


def _build_kernel():
    nc = bass.Bass("TRN2", target_bir_lowering=False, debug=False,
                   num_devices=N_CORES)

    xt = nc.dram_tensor("xt", [128, KD, S], F16, kind="ExternalInput")
    xr = nc.dram_tensor("xr", [S, D], F32, kind="ExternalInput")
    w1t = nc.dram_tensor("w1t", [TOPK, KF, 128, KD, 128], F16,
                         kind="ExternalInput")
    w2t = nc.dram_tensor("w2t", [TOPK, 128, KF, D], F16, kind="ExternalInput")
    b1t = nc.dram_tensor("b1t", [128, TOPK, KF], F32, kind="ExternalInput")
    b2c = nc.dram_tensor("b2c", [128, D], F32, kind="ExternalInput")
    gam = nc.dram_tensor("gam", [128, D], F32, kind="ExternalInput")
    out = nc.dram_tensor("out", [S, D], F32, kind="ExternalOutput")

    gelu = mybir.ActivationFunctionType.Gelu_apprx_tanh

    with tile.TileContext(nc) as tc:
        with (
            tc.tile_pool(name="consts", bufs=1) as consts,
            tc.tile_pool(name="w1p", bufs=12) as w1p,
            tc.tile_pool(name="w2p", bufs=8) as w2p,
            tc.tile_pool(name="ps", bufs=4, space="PSUM") as psp,
            tc.tile_pool(name="xrp", bufs=4) as xrp,
            tc.tile_pool(name="mixp", bufs=2) as mixp,
            tc.tile_pool(name="outp", bufs=3) as outp,
            tc.tile_pool(name="stat", bufs=4) as stat,
        ):
            # ---- constants / persistent tiles ----
            xt_sb = consts.tile([128, KD, S], F16)
            b1_sb = consts.tile([128, TOPK, KF], F32)
            # b2c/gam are only needed by the LayerNorm stage — allocated here,
            # but their DMAs are issued mid-GEMM1 (below) so they don't delay
            # the startup-critical w1 stream on the gpsimd queue.
            b2c_sb = consts.tile([128, D], F32)
            gam_sb = consts.tile([128, D], F32)
            eps_sb = consts.tile([128, 1], F32)
            nc.vector.memset(eps_sb[:], LN_EPS)
            # g = gelu(x @ W1 + b1), transposed layout [ff, s-half]
            gt = consts.tile([128, TOPK, KF, SH], F16)

            for half in range(2):
                s0 = half * SH

                # ---- GEMM1: h.T = W1.T-tiles @ x.T, fused bias+gelu ----
                # m-tiles processed in pairs with MMs interleaved between two
                # PSUM tiles: consecutive matmuls then hit different banks,
                # which lets fill/drain overlap (same-bank back-to-back
                # accumulation serializes on the bank write port).
                for e in range(TOPK):
                    for m in range(0, KF, 2):
                        # the first few tiles of the kernel ride the (idle,
                        # low-latency) HWDGE queue so GEMM1 starts promptly;
                        # the steady stream stays on SWDGE
                        weng = (nc.sync if half == 0 and e == 0 and m < 8
                                else nc.gpsimd)
                        w1a = w1p.tile([128, KD, 128], F16, tag="w1")
                        weng.dma_start(w1a[:], w1t[e, m])
                        w1b = w1p.tile([128, KD, 128], F16, tag="w1")
                        weng.dma_start(w1b[:], w1t[e, m + 1])
                        if half == 0 and e == 0 and m == 0:
                            for kd in range(KD):
                                nc.sync.dma_start(
                                    xt_sb[:, kd, 0:SH], xt[:, kd, 0:SH])
                            nc.sync.dma_start(b1_sb[:], b1t[:])
                        if half == 0 and e == 0 and m == 2:
                            for kd in range(KD):
                                nc.sync.dma_start(
                                    xt_sb[:, kd, SH:S], xt[:, kd, SH:S])
                        pha_t = psp.tile([128, 2 * SH], F32, tag="ps")
                        phb_t = psp.tile([128, 2 * SH], F32, tag="ps")
                        pha = pha_t[:, :SH]
                        phb = phb_t[:, :SH]
                        for kd in range(KD):
                            nc.tensor.matmul(
                                pha,
                                w1a[:, kd, :],
                                xt_sb[:, kd, s0 : s0 + SH],
                                start=(kd == 0),
                                stop=(kd == KD - 1),
                            )
                            nc.tensor.matmul(
                                phb,
                                w1b[:, kd, :],
                                xt_sb[:, kd, s0 : s0 + SH],
                                start=(kd == 0),
                                stop=(kd == KD - 1),
                            )
                        nc.scalar.activation(
                            out=gt[:, e, m, :],
                            in_=pha,
                            func=gelu,
                            bias=b1_sb[:, e, m : m + 1],
                            scale=1.0,
                        )
                        nc.scalar.activation(
                            out=gt[:, e, m + 1, :],
                            in_=phb,
                            func=gelu,
                            bias=b1_sb[:, e, m + 1 : m + 2],
                            scale=1.0,
                        )
                        if half == 0 and e == 0 and m == 8:
                            # LN constants: sync queue is idle mid-GEMM1
                            nc.sync.dma_start(b2c_sb[:], b2c[:])
                            nc.sync.dma_start(gam_sb[:], gam[:])

                # ---- GEMM2: mixed[s,d] accumulates both experts ----
                # One [128, 1024] PSUM tile (2 banks) per s-chunk: cols 0:512
                # (bank A) and 512:768 (bank B) are separate accumulation
                # groups; LN later reads 0:768 as one contiguous slab.
                pm = [psp.tile([128, 2 * SH], F32, tag="ps", name=f"pm_{half}_{i}")
                      for i in range(SC)]
                xr_sbs = []
                for sc in range(SC):
                    srow = s0 + sc * 128
                    xr_sb = xrp.tile([128, D], F32, tag="xr")
                    nc.gpsimd.dma_start(xr_sb[:], xr[srow : srow + 128, :])
                    xr_sbs.append(xr_sb)
                for e in range(TOPK):
                    for kf in range(KF):
                        w2_sb = w2p.tile([128, D], F16, tag="w2")
                        nc.sync.dma_start(w2_sb[:], w2t[e, :, kf, :])
                        first = e == 0 and kf == 0
                        last = e == TOPK - 1 and kf == KF - 1
                        for sc in range(SC):
                            lhsT = gt[:, e, kf, sc * 128 : (sc + 1) * 128]
                            nc.tensor.matmul(
                                pm[sc][:, 0:512],
                                lhsT,
                                w2_sb[:, 0:512],
                                start=first,
                                stop=last,
                            )
                            nc.tensor.matmul(
                                pm[sc][:, 512:768],
                                lhsT,
                                w2_sb[:, 512:768],
                                start=first,
                                stop=last,
                            )

                # ---- combine + LayerNorm + residual, per s-chunk ----
                for sc in range(SC):
                    srow = s0 + sc * 128
                    xr_sb = xr_sbs[sc]
                    mixed = mixp.tile([128, D], F32, tag="mx")
                    nc.vector.tensor_add(mixed[:], pm[sc][:, 0:768],
                                         b2c_sb[:])

                    stats = stat.tile([128, 3, 6], F32, tag="st")
                    for i in range(3):
                        nc.vector.bn_stats(
                            out=stats[:, i, :],
                            in_=mixed[:, i * 256 : (i + 1) * 256],
                        )
                    mv = stat.tile([128, 2], F32, tag="mv")
                    nc.vector.bn_aggr(out=mv[:], in_=stats[:])

                    rs = stat.tile([128, 1], F32, tag="rs")
                    nc.scalar.activation(
                        out=rs[:], in_=mv[:, 1:2],
                        func=mybir.ActivationFunctionType.Sqrt,
                        bias=eps_sb[:], scale=1.0,
                    )
                    nc.vector.reciprocal(out=rs[:], in_=rs[:])

                    # (mixed - mu) * rs, then * gamma, then + (x + beta)
                    nc.vector.tensor_scalar(
                        out=mixed[:],
                        in0=mixed[:],
                        scalar1=mv[:, 0:1],
                        scalar2=rs[:],
                        op0=mybir.AluOpType.subtract,
                        op1=mybir.AluOpType.mult,
                    )
                    o_sb = outp.tile([128, D], F32, tag="o")
                    nc.vector.tensor_mul(o_sb[:], mixed[:], gam_sb[:])
                    nc.vector.tensor_add(o_sb[:], o_sb[:], xr_sb[:])
                    nc.sync.dma_start(out[srow : srow + 128, :], o_sb[:])

    _split_multi_waits(nc)
    return nc


_NC_CACHE = None


def _get_nc():
    global _NC_CACHE
    if _NC_CACHE is None:
        _NC_CACHE = _build_kernel()
    return _NC_CACHE


def _route(x, text_state, router_w, router_b):
    """Host router replicating the jax fp32 ops."""
    x = x.astype(np.float32, copy=False)
    pooled = x.mean(axis=1, dtype=np.float32)                 # [B, D]
    feat = np.concatenate([pooled, text_state.astype(np.float32)], axis=-1)
    logits = feat @ router_w.astype(np.float32) + router_b.astype(np.float32)
    m = logits.max(axis=-1, keepdims=True)
    p = np.exp(logits - m)
    probs = (p / p.sum(axis=-1, keepdims=True)).astype(np.float32)
    # jax.lax.top_k: descending, ties -> lower index first
    topi = np.argsort(-probs, axis=-1, kind="stable")[:, :TOPK]
    topv = np.take_along_axis(probs, topi, axis=-1)
    return probs, topi, topv


def kernel(x, text_state, W1, b1, W2, b2, router_w, router_b, ln_gamma,
           ln_beta):
    x = np.asarray(x, dtype=np.float32)
    text_state = np.asarray(text_state, dtype=np.float32)
    W1 = np.asarray(W1, dtype=np.float32)
    b1 = np.asarray(b1, dtype=np.float32)
    W2 = np.asarray(W2, dtype=np.float32)
    b2 = np.asarray(b2, dtype=np.float32)
    router_w = np.asarray(router_w, dtype=np.float32)
    router_b = np.asarray(router_b, dtype=np.float32)
    ln_gamma = np.asarray(ln_gamma, dtype=np.float32)
    ln_beta = np.asarray(ln_beta, dtype=np.float32)

    probs, topi, topv = _route(x, text_state, router_w, router_b)

    gam_b = np.ascontiguousarray(np.broadcast_to(ln_gamma, (128, D)))

    in_maps = []
    for bidx in range(B):
        xb = x[bidx]                                          # [S, D]
        # x.T tiled: xt[p, kd, s] = x[s, kd*128+p]
        xt = np.ascontiguousarray(
            xb.T.reshape(KD, 128, S).transpose(1, 0, 2)).astype(np.float16)
        xr = np.ascontiguousarray(xb + ln_beta)               # residual + beta
        eids = topi[bidx]
        # w1t[e, m, p, kd, j] = W1[eid, kd*128+p, m*128+j]
        w1g = W1[eids]                                        # [2, D, FF]
        w1t = np.ascontiguousarray(
            w1g.reshape(TOPK, KD, 128, KF, 128).transpose(0, 3, 2, 1, 4)
        ).astype(np.float16)
        # w2t[e, p, kf, d] = topv_e * W2[eid, kf*128+p, d]
        w2g = W2[eids] * topv[bidx][:, None, None]            # [2, FF, D]
        w2t = np.ascontiguousarray(
            w2g.reshape(TOPK, KF, 128, D).transpose(0, 2, 1, 3)
        ).astype(np.float16)
        # b1t[p, e, m] = b1[eid, m*128+p]
        b1g = b1[eids].reshape(TOPK, KF, 128)                 # [2, KF, 128]
        b1t = np.ascontiguousarray(b1g.transpose(2, 0, 1))
        # combined, topv-weighted b2, broadcast over partitions
        b2cv = (topv[bidx][:, None] * b2[eids]).sum(axis=0)   # [D]
        b2cb = np.ascontiguousarray(np.broadcast_to(b2cv, (128, D)))

        in_maps.append({
            "xt": xt, "xr": xr, "w1t": w1t, "w2t": w2t,
            "b1t": b1t, "b2c": b2cb, "gam": gam_b,
        })

    global _last_in_maps
    _last_in_maps = in_maps

    nc = _get_nc()
    res = run_bass_kernel_spmd(nc, in_maps, core_ids=list(range(N_CORES)))
    out = np.stack([res.results[bidx]["out"] for bidx in range(B)], axis=0)
    return out, probs


if __name__ == "__main__":
    rng = np.random.default_rng(0)
    inputs = {
        "x": rng.standard_normal((B, S, D), dtype=np.float32),
        "text_state": rng.standard_normal((B, D), dtype=np.float32),
        "W1": rng.standard_normal((E, D, FF), dtype=np.float32) * 0.02,
        "b1": np.zeros((E, FF), np.float32),
        "W2": rng.standard_normal((E, FF, D), dtype=np.float32) * 0.02,
        "b2": np.zeros((E, D), np.float32),
        "router_w": rng.standard_normal((2 * D, E), dtype=np.float32) * 0.02,
        "router_b": np.zeros((E,), np.float32),
        "ln_gamma": np.ones((D,), np.float32),
        "ln_beta": np.zeros((D,), np.float32),
    }
    o, pr = kernel(**inputs)
    print(o.shape, pr.shape, o.dtype, pr.dtype)
